# revision 9
# baseline (speedup 1.0000x reference)
"""Haar DWT (single-level, separable) Trainium2 Bass kernel.

Input  x: (64, 1, 1024, 1024) fp32
Output  : (64, 4, 512, 512) fp32 — channels [LL, LH, HL, HH] (pywt convention)

Strategy: pure data parallel — 8 images per NeuronCore, 8 cores.
Per core, per image (1024x1024):
  - process 8 chunks of 128 consecutive rows
  - vertical Haar butterfly on the TensorEngine: a 128x128 banded matrix W
    (W[2i,2i]=W[2i+1,2i]=0.5, W[2i,2i+1]=-0.5, W[2i+1,2i+1]=0.5) maps the
    128 input rows (partitions) to 64 interleaved (sum, diff) row pairs
  - horizontal butterfly on the VectorEngine straight out of PSUM:
      accA = ps[:, 0::2] + ps[:, 1::2]   (even partitions: LL, odd: LH)
      accB = ps[:, 1::2] - ps[:, 0::2]   (even partitions: HL, odd: HH)
  - outputs accumulate in SBUF per image, then 4 x 1MB DMAs to DRAM
"""

import os
import sys

import numpy as np

for _p in (
    "/root/.axon_site",
    "/root/.axon_site/_ro/trn_rl_repo",
    "/root/.axon_site/_ro/pypackages",
    "/opt/trn_rl_repo",
):
    if os.path.isdir(_p) and _p not in sys.path:
        sys.path.append(_p)

from concourse import bacc, bass, mybir, tile  # noqa: E402
from concourse.bass_utils import run_bass_kernel_spmd  # noqa: E402

N_CORES = 8
IMG_PER_CORE = 8
H = 1024
W = 1024
ROWS_PER_CHUNK = 128
N_CHUNKS = H // ROWS_PER_CHUNK  # 8
OUT_ROWS_PER_CHUNK = ROWS_PER_CHUNK // 2  # 64
HW_OUT = H // 2  # 512
WW_OUT = W // 2  # 512
F32 = mybir.dt.float32


def _butterfly_matrix() -> np.ndarray:
    """W[k, m] = coefficient of input row k in output partition m.
    m<64:  0.5*(row 2m + row 2m+1)        (vertical low-pass, partitions 0:64)
    m>=64: 0.5*(row 2i+1 - row 2i), i=m-64 (vertical high-pass, partitions 64:128)
    Grouped (not interleaved) so each output channel lands in a dense
    partition block after the horizontal butterfly."""
    Wm = np.zeros((128, 128), dtype=np.float32)
    for i in range(64):
        Wm[2 * i, i] = 0.5
        Wm[2 * i + 1, i] = 0.5
        Wm[2 * i, 64 + i] = -0.5
        Wm[2 * i + 1, 64 + i] = 0.5
    return Wm


def build_program(n_img: int = IMG_PER_CORE) -> bass.Bass:
    # Bacc (not plain Bass): its compile() runs move_matmul_waits_to_ldweights
    # + generate_event_semaphores, which split multi-sem waits down to the
    # 1-wait-per-instruction TRN2 limit that walrus codegen enforces.
    nc = bacc.Bacc(
        "TRN2",
        target_bir_lowering=False,
        debug=False,
        num_devices=N_CORES,
    )
    x_d = nc.dram_tensor("x", [n_img, H, W], F32, kind="ExternalInput")
    w_d = nc.dram_tensor("w", [128, 128], F32, kind="ExternalInput")
    o_d = nc.dram_tensor("out", [n_img, 4, HW_OUT, WW_OUT], F32, kind="ExternalOutput")

    with tile.TileContext(nc) as tc:
        with (
            tc.tile_pool(name="wpool", bufs=1) as wpool,
            tc.tile_pool(name="inpool", bufs=4) as inpool,
            tc.tile_pool(name="hpool", bufs=4) as hpool,
            tc.tile_pool(name="psum", bufs=4, space="PSUM") as psumpool,
            tc.tile_pool(name="apool", bufs=2) as apool,
            tc.tile_pool(name="bpool", bufs=2) as bpool,
        ):
            wt = wpool.tile([128, 128], F32)
            nc.sync.dma_start(out=wt[:], in_=w_d[:])

            for img in range(n_img):
                # partitions 0:64 of accA: LL rows, 64:128: LH rows
                # partitions 0:64 of accB: HL rows, 64:128: HH rows
                accA = apool.tile([128, N_CHUNKS * WW_OUT], F32)
                accB = bpool.tile([128, N_CHUNKS * WW_OUT], F32)
                for q in range(N_CHUNKS):
                    xt = inpool.tile([128, W], F32)
                    nc.sync.dma_start(
                        out=xt[:],
                        in_=x_d[img, q * ROWS_PER_CHUNK : (q + 1) * ROWS_PER_CHUNK, :],
                    )
                    # horizontal butterfly (DVE, SBUF->SBUF)
                    h1 = hpool.tile([128, WW_OUT], F32)
                    h2 = hpool.tile([128, WW_OUT], F32)
                    nc.vector.tensor_add(out=h1[:], in0=xt[:, 0::2], in1=xt[:, 1::2])
                    nc.vector.tensor_sub(out=h2[:], in0=xt[:, 1::2], in1=xt[:, 0::2])
                    # vertical butterfly (PE): W groups sums into partitions
                    # 0:64 and diffs into 64:128, with the 0.5 scale folded in
                    psA = psumpool.tile([128, WW_OUT], F32)
                    psB = psumpool.tile([128, WW_OUT], F32)
                    nc.tensor.matmul(psA[:], wt[:], h1[:])
                    nc.tensor.matmul(psB[:], wt[:], h2[:])
                    # PSUM -> SBUF accumulation (ScalarE; DVE is busier)
                    col = q * WW_OUT
                    nc.scalar.copy(out=accA[:, col : col + WW_OUT], in_=psA[:])
                    nc.scalar.copy(out=accB[:, col : col + WW_OUT], in_=psB[:])
                for ch, (acc, lo) in enumerate(
                    [(accA, 0), (accA, 64), (accB, 0), (accB, 64)]
                ):
                    src = acc[lo : lo + 64, :].rearrange("i (q c) -> i q c", c=WW_OUT)
                    dst = o_d[img, ch].rearrange("(q i) c -> i q c", q=N_CHUNKS)
                    nc.sync.dma_start(out=dst, in_=src)
    nc.compile()
    return nc


_PROGRAM_CACHE: dict[int, bass.Bass] = {}


def _program(n_img: int) -> bass.Bass:
    if n_img not in _PROGRAM_CACHE:
        _PROGRAM_CACHE[n_img] = build_program(n_img)
    return _PROGRAM_CACHE[n_img]


def run(x: np.ndarray, trace: bool = False, **spmd_kwargs):
    """x: (B, 1, H, W) fp32 -> (B, 4, H/2, W/2) fp32.
    Returns (output, BassKernelResults)."""
    B = x.shape[0]
    assert x.shape == (B, 1, H, W), x.shape
    assert B % N_CORES == 0
    n_img = B // N_CORES
    nc = _program(n_img)
    wm = _butterfly_matrix()
    x3 = np.ascontiguousarray(x[:, 0], dtype=np.float32)  # (B, H, W)
    in_maps = [
        {"x": x3[i * n_img : (i + 1) * n_img], "w": wm} for i in range(N_CORES)
    ]
    res = run_bass_kernel_spmd(
        nc, in_maps, core_ids=list(range(N_CORES)), trace=trace, **spmd_kwargs
    )
    out = np.concatenate([r["out"] for r in res.results], axis=0)
    return out.astype(np.float32, copy=False), res


def kernel(x: np.ndarray) -> np.ndarray:
    out, _ = run(np.asarray(x))
    return out


# revision 14
# speedup vs baseline: 1.1746x; 1.1746x over previous
"""Haar DWT (single-level, separable) Trainium2 Bass kernel.

Input  x: (64, 1, 1024, 1024) fp32
Output  : (64, 4, 512, 512) fp32 — channels [LL, LH, HL, HH] (pywt convention)

Strategy: pure data parallel — 8 images per NeuronCore, 8 cores.
Per core, per image (1024x1024):
  - one 4MB input DMA: partition p holds rows {t*128+p, t=0..7} (sync HWDGE ring)
  - per 128-row chunk t:
      horizontal butterfly on DVE (SBUF->SBUF, stride-2 column reads):
        h1 = x_even_cols + x_odd_cols,  h2 = x_odd_cols - x_even_cols
      vertical butterfly on the TensorEngine: a 128x128 banded matrix W
      (0.5-scaled, sums grouped into partitions 0:64, diffs into 64:128)
        psA = W.T @ h1  -> LL rows in partitions 0:64, LH rows in 64:128
        psB = W.T @ h2  -> HL rows in partitions 0:64, HH rows in 64:128
      PSUM -> SBUF accumulation copies on ScalarE
  - two 2MB output DMAs per image (channel pairs share one full
    128-partition transfer), issued on the scalar HWDGE ring so input and
    output streams ride different rings.
"""

import os
import sys

import numpy as np

for _p in (
    "/root/.axon_site",
    "/root/.axon_site/_ro/trn_rl_repo",
    "/root/.axon_site/_ro/pypackages",
    "/opt/trn_rl_repo",
):
    if os.path.isdir(_p) and _p not in sys.path:
        sys.path.append(_p)

from concourse import bacc, bass, mybir, tile  # noqa: E402
from concourse.bass_utils import run_bass_kernel_spmd  # noqa: E402

N_CORES = 8
IMG_PER_CORE = 8
H = 1024
W = 1024
ROWS_PER_CHUNK = 128
N_CHUNKS = H // ROWS_PER_CHUNK  # 8
HW_OUT = H // 2  # 512
WW_OUT = W // 2  # 512
F32 = mybir.dt.float32
F32R = mybir.dt.float32r


def _butterfly_matrix() -> np.ndarray:
    """W[k, m] = coefficient of input row k in output partition m.
    m<64:  0.5*(row 2m + row 2m+1)        (vertical low-pass, partitions 0:64)
    m>=64: 0.5*(row 2i+1 - row 2i), i=m-64 (vertical high-pass, 64:128)."""
    Wm = np.zeros((128, 128), dtype=np.float32)
    for i in range(64):
        Wm[2 * i, i] = 0.5
        Wm[2 * i + 1, i] = 0.5
        Wm[2 * i, 64 + i] = -0.5
        Wm[2 * i + 1, 64 + i] = 0.5
    return Wm


def build_program(n_img: int = IMG_PER_CORE, use_f32r: bool = True) -> bass.Bass:
    # Bacc (not plain Bass): its compile() runs move_matmul_waits_to_ldweights
    # + generate_event_semaphores, which split multi-sem waits down to the
    # 1-wait-per-instruction TRN2 limit that walrus codegen enforces.
    nc = bacc.Bacc(
        "TRN2",
        target_bir_lowering=False,
        debug=False,
        num_devices=N_CORES,
    )
    x_d = nc.dram_tensor("x", [n_img, H, W], F32, kind="ExternalInput")
    w_d = nc.dram_tensor("w", [128, 128], F32, kind="ExternalInput")
    o_d = nc.dram_tensor("out", [n_img, 4, HW_OUT, WW_OUT], F32, kind="ExternalOutput")

    mm_dt = F32R if use_f32r else F32

    with tile.TileContext(nc) as tc:
        with (
            tc.tile_pool(name="wpool", bufs=1) as wpool,
            tc.tile_pool(name="inpool", bufs=2) as inpool,
            tc.tile_pool(name="hpool", bufs=4) as hpool,
            tc.tile_pool(name="psum", bufs=4, space="PSUM") as psumpool,
            tc.tile_pool(name="apool", bufs=2) as apool,
            tc.tile_pool(name="bpool", bufs=2) as bpool,
        ):
            wt_raw = wpool.tile([128, 128], F32)
            nc.sync.dma_start(out=wt_raw[:], in_=w_d[:])
            if use_f32r:
                # PE weights must be f32r-rounded; +-0.5 entries are exact
                wt = wpool.tile([128, 128], F32R)
                nc.vector.tensor_copy(out=wt[:], in_=wt_raw[:])
            else:
                wt = wt_raw

            for img in range(n_img):
                # one 4MB contiguous-DRAM load: partition p <- rows t*128+p
                # (SWDGE queue row; HWDGE rings are kept for the stores)
                xt = inpool.tile([128, N_CHUNKS, W], F32)
                nc.gpsimd.dma_start(
                    out=xt[:],
                    in_=x_d[img].rearrange("(t p) c -> p t c", p=128),
                )
                # accA partitions 0:64: LL rows, 64:128: LH rows
                # accB partitions 0:64: HL rows, 64:128: HH rows
                accA = apool.tile([128, N_CHUNKS * WW_OUT], F32)
                accB = bpool.tile([128, N_CHUNKS * WW_OUT], F32)
                for t in range(N_CHUNKS):
                    xc = xt[:, t, :]
                    h1 = hpool.tile([128, WW_OUT], mm_dt)
                    h2 = hpool.tile([128, WW_OUT], mm_dt)
                    nc.vector.tensor_add(out=h1[:], in0=xc[:, 0::2], in1=xc[:, 1::2])
                    nc.vector.tensor_sub(out=h2[:], in0=xc[:, 1::2], in1=xc[:, 0::2])
                    psA = psumpool.tile([128, WW_OUT], F32)
                    psB = psumpool.tile([128, WW_OUT], F32)
                    nc.tensor.matmul(psA[:], wt[:], h1[:])
                    nc.tensor.matmul(psB[:], wt[:], h2[:])
                    col = t * WW_OUT
                    nc.scalar.copy(out=accA[:, col : col + WW_OUT], in_=psA[:])
                    nc.scalar.copy(out=accB[:, col : col + WW_OUT], in_=psB[:])
                # four 1MB stores; each HWDGE ring gets one even-engine
                # (partitions 0:64) and one odd-engine (64:128) DMA so all 16
                # SDMA engines stay busy on both rings
                for ch, acc, lo, eng in (
                    (0, accA, 0, nc.sync),  # LL
                    (1, accA, 64, nc.scalar),  # LH
                    (2, accB, 0, nc.scalar),  # HL
                    (3, accB, 64, nc.sync),  # HH
                ):
                    src = acc[lo : lo + 64, :].rearrange("i (t c) -> i t c", c=WW_OUT)
                    dst = o_d[img, ch].rearrange("(t i) c -> i t c", t=N_CHUNKS)
                    eng.dma_start(out=dst, in_=src)
    nc.compile()
    return nc


_PROGRAM_CACHE: dict[tuple, bass.Bass] = {}


def _program(n_img: int, use_f32r: bool = True) -> bass.Bass:
    key = (n_img, use_f32r)
    if key not in _PROGRAM_CACHE:
        _PROGRAM_CACHE[key] = build_program(n_img, use_f32r)
    return _PROGRAM_CACHE[key]


def run(x: np.ndarray, trace: bool = False, use_f32r: bool = True, **spmd_kwargs):
    """x: (B, 1, H, W) fp32 -> (B, 4, H/2, W/2) fp32.
    Returns (output, BassKernelResults)."""
    B = x.shape[0]
    assert x.shape == (B, 1, H, W), x.shape
    assert B % N_CORES == 0
    n_img = B // N_CORES
    nc = _program(n_img, use_f32r)
    wm = _butterfly_matrix()
    x3 = np.ascontiguousarray(x[:, 0], dtype=np.float32)  # (B, H, W)
    in_maps = [
        {"x": x3[i * n_img : (i + 1) * n_img], "w": wm} for i in range(N_CORES)
    ]
    res = run_bass_kernel_spmd(
        nc, in_maps, core_ids=list(range(N_CORES)), trace=trace, **spmd_kwargs
    )
    out = np.concatenate([r["out"] for r in res.results], axis=0)
    return out.astype(np.float32, copy=False), res


def kernel(x: np.ndarray) -> np.ndarray:
    out, _ = run(np.asarray(x))
    return out


# revision 21
# speedup vs baseline: 1.4191x; 1.2081x over previous
"""Haar DWT (single-level, separable) Trainium2 Bass kernel.

Input  x: (64, 1, 1024, 1024) fp32
Output  : (64, 4, 512, 512) fp32 — channels [LL, LH, HL, HH] (pywt convention)

Strategy: pure data parallel — 8 images per NeuronCore, 8 cores.
Per core, per image (1024x1024):
  - one 4MB input DMA: partition p holds rows {t*128+p, t=0..7} (sync HWDGE ring)
  - per 128-row chunk t:
      horizontal butterfly on DVE (SBUF->SBUF, stride-2 column reads):
        h1 = x_even_cols + x_odd_cols,  h2 = x_odd_cols - x_even_cols
      vertical butterfly on the TensorEngine: a 128x128 banded matrix W
      (0.5-scaled, sums grouped into partitions 0:64, diffs into 64:128)
        psA = W.T @ h1  -> LL rows in partitions 0:64, LH rows in 64:128
        psB = W.T @ h2  -> HL rows in partitions 0:64, HH rows in 64:128
      PSUM -> SBUF accumulation copies on ScalarE
  - two 2MB output DMAs per image (channel pairs share one full
    128-partition transfer), issued on the scalar HWDGE ring so input and
    output streams ride different rings.
"""

import os
import sys

import numpy as np

for _p in (
    "/root/.axon_site",
    "/root/.axon_site/_ro/trn_rl_repo",
    "/root/.axon_site/_ro/pypackages",
    "/opt/trn_rl_repo",
):
    if os.path.isdir(_p) and _p not in sys.path:
        sys.path.append(_p)

from concourse import bacc, bass, mybir, tile  # noqa: E402
from concourse.bass_utils import run_bass_kernel_spmd  # noqa: E402

N_CORES = 8
IMG_PER_CORE = 8
H = 1024
W = 1024
ROWS_PER_CHUNK = 128
N_CHUNKS = H // ROWS_PER_CHUNK  # 8
HW_OUT = H // 2  # 512
WW_OUT = W // 2  # 512
F32 = mybir.dt.float32
F32R = mybir.dt.float32r


def _butterfly_matrix() -> np.ndarray:
    """W[k, m] = coefficient of input row k in output partition m.
    m<64:  0.5*(row 2m + row 2m+1)        (vertical low-pass, partitions 0:64)
    m>=64: 0.5*(row 2i+1 - row 2i), i=m-64 (vertical high-pass, 64:128)."""
    Wm = np.zeros((128, 128), dtype=np.float32)
    for i in range(64):
        Wm[2 * i, i] = 0.5
        Wm[2 * i + 1, i] = 0.5
        Wm[2 * i, 64 + i] = -0.5
        Wm[2 * i + 1, 64 + i] = 0.5
    return Wm


def _butterfly_matrices_pm() -> np.ndarray:
    """[W | -W] side by side, (128, 256)."""
    Wm = _butterfly_matrix()
    return np.concatenate([Wm, -Wm], axis=1)


def build_program(
    n_img: int = IMG_PER_CORE, use_f32r: bool = True, direct_mm: bool = True
) -> bass.Bass:
    # Bacc (not plain Bass): its compile() runs move_matmul_waits_to_ldweights
    # + generate_event_semaphores, which split multi-sem waits down to the
    # 1-wait-per-instruction TRN2 limit that walrus codegen enforces.
    nc = bacc.Bacc(
        "TRN2",
        target_bir_lowering=False,
        debug=False,
        num_devices=N_CORES,
    )
    mm_dt = F32R if use_f32r else F32
    in_dt = mm_dt if direct_mm else F32

    x_d = nc.dram_tensor("x", [n_img, H, W], F32, kind="ExternalInput")
    w_d = nc.dram_tensor("w", [128, 256], F32, kind="ExternalInput")
    o_d = nc.dram_tensor("out", [n_img, 4, HW_OUT, WW_OUT], F32, kind="ExternalOutput")

    with tile.TileContext(nc) as tc:
        with (
            tc.tile_pool(name="wpool", bufs=1) as wpool,
            tc.tile_pool(name="inpool", bufs=4) as inpool,
            tc.tile_pool(name="hpool", bufs=4) as hpool,
            tc.tile_pool(name="psum", bufs=4, space="PSUM") as psumpool,
            tc.tile_pool(name="apool", bufs=2) as apool,
            tc.tile_pool(name="bpool", bufs=2) as bpool,
        ):
            wt_raw = wpool.tile([128, 256], F32)
            nc.sync.dma_start(out=wt_raw[:], in_=w_d[:])
            if use_f32r:
                # PE weights must be f32r-rounded; +-0.5 entries are exact
                wt_all = wpool.tile([128, 256], F32R)
                nc.vector.tensor_copy(out=wt_all[:], in_=wt_raw[:])
            else:
                wt_all = wt_raw
            wt = wt_all[:, 0:128]  # W
            wtn = wt_all[:, 128:256]  # -W

            for img in range(n_img):
                # two 2MB contiguous-DRAM loads: partition p <- rows t*128+p
                # (SWDGE queue row — it can also cast f32 -> f32r in flight;
                # HWDGE rings are kept for the stores)
                halves = []
                for hv in range(2):
                    xh = inpool.tile([128, N_CHUNKS // 2, W], in_dt)
                    nc.gpsimd.dma_start(
                        out=xh[:],
                        in_=x_d[img, hv * (H // 2) : (hv + 1) * (H // 2)].rearrange(
                            "(t p) c -> p t c", p=128
                        ),
                    )
                    halves.append(xh)
                # accA partitions 0:64: LL rows, 64:128: LH rows
                # accB partitions 0:64: HL rows, 64:128: HH rows
                accA = apool.tile([128, N_CHUNKS * WW_OUT], F32)
                accB = bpool.tile([128, N_CHUNKS * WW_OUT], F32)
                for t in range(N_CHUNKS):
                    xc = halves[t // (N_CHUNKS // 2)][:, t % (N_CHUNKS // 2), :]
                    psA = psumpool.tile([128, WW_OUT], F32)
                    psB = psumpool.tile([128, WW_OUT], F32)
                    if direct_mm:
                        # horizontal butterfly via PSUM accumulation:
                        #   psA = W.T@x_even + W.T@x_odd   (LL | LH rows)
                        #   psB = -W.T@x_even + W.T@x_odd  (HL | HH rows)
                        xe, xo = xc[:, 0::2], xc[:, 1::2]
                        nc.tensor.matmul(psA[:], wt, xe, start=True, stop=False)
                        nc.tensor.matmul(psA[:], wt, xo, start=False, stop=True)
                        nc.tensor.matmul(psB[:], wtn, xe, start=True, stop=False)
                        nc.tensor.matmul(psB[:], wt, xo, start=False, stop=True)
                    else:
                        h1 = hpool.tile([128, WW_OUT], mm_dt)
                        h2 = hpool.tile([128, WW_OUT], mm_dt)
                        nc.vector.tensor_add(
                            out=h1[:], in0=xc[:, 0::2], in1=xc[:, 1::2]
                        )
                        nc.vector.tensor_sub(
                            out=h2[:], in0=xc[:, 1::2], in1=xc[:, 0::2]
                        )
                        nc.tensor.matmul(psA[:], wt, h1[:])
                        nc.tensor.matmul(psB[:], wt, h2[:])
                    col = t * WW_OUT
                    nc.scalar.copy(out=accA[:, col : col + WW_OUT], in_=psA[:])
                    nc.scalar.copy(out=accB[:, col : col + WW_OUT], in_=psB[:])
                # four 1MB stores; each HWDGE ring gets one even-engine
                # (partitions 0:64) and one odd-engine (64:128) DMA so all 16
                # SDMA engines stay busy on both rings
                for ch, acc, lo, eng in (
                    (0, accA, 0, nc.sync),  # LL
                    (1, accA, 64, nc.scalar),  # LH
                    (2, accB, 0, nc.scalar),  # HL
                    (3, accB, 64, nc.sync),  # HH
                ):
                    src = acc[lo : lo + 64, :].rearrange("i (t c) -> i t c", c=WW_OUT)
                    dst = o_d[img, ch].rearrange("(t i) c -> i t c", t=N_CHUNKS)
                    eng.dma_start(out=dst, in_=src)
    nc.compile()
    return nc


_PROGRAM_CACHE: dict[tuple, bass.Bass] = {}


def _program(n_img: int, use_f32r: bool = True, direct_mm: bool = True) -> bass.Bass:
    key = (n_img, use_f32r, direct_mm)
    if key not in _PROGRAM_CACHE:
        _PROGRAM_CACHE[key] = build_program(n_img, use_f32r, direct_mm)
    return _PROGRAM_CACHE[key]


def run(
    x: np.ndarray,
    trace: bool = False,
    use_f32r: bool = True,
    direct_mm: bool = True,
    **spmd_kwargs,
):
    """x: (B, 1, H, W) fp32 -> (B, 4, H/2, W/2) fp32.
    Returns (output, BassKernelResults)."""
    B = x.shape[0]
    assert x.shape == (B, 1, H, W), x.shape
    assert B % N_CORES == 0
    n_img = B // N_CORES
    nc = _program(n_img, use_f32r, direct_mm)
    wm = _butterfly_matrices_pm()
    x3 = np.ascontiguousarray(x[:, 0], dtype=np.float32)  # (B, H, W)
    in_maps = [
        {"x": x3[i * n_img : (i + 1) * n_img], "w": wm} for i in range(N_CORES)
    ]
    try:
        res = run_bass_kernel_spmd(
            nc, in_maps, core_ids=list(range(N_CORES)), trace=trace, **spmd_kwargs
        )
    except Exception:
        # transient NRT device errors have been observed; retry once
        import time

        time.sleep(2.0)
        res = run_bass_kernel_spmd(
            nc, in_maps, core_ids=list(range(N_CORES)), trace=trace, **spmd_kwargs
        )
    out = np.concatenate([r["out"] for r in res.results], axis=0)
    return out.astype(np.float32, copy=False), res


def kernel(x: np.ndarray) -> np.ndarray:
    out, _ = run(np.asarray(x))
    return out
